# revision 9
# baseline (speedup 1.0000x reference)
"""Trainium2 Bass kernel for ContrastiveMSELoss.

Reference computes, over all N^2 pairs (diagonal masked to 0):
    mse_ij  = (|x_i|^2 + |x_j|^2 - 2 x_i.x_j) / D
    sign_ij = +1 if class_i == class_j else -1
    loss    = mean_ij(sign_ij * mse_ij) + BETA

Using sum_{i,j in c} x_i.x_j = |M_c|^2 with M_c = sum_{i in c} x_i, the
loss collapses to class-bucketed first/second moments (O(N*D) work,
memory-bound -- no N x N gram matrix needed):

    T_same = sum_c (2 n_c SQ_c - 2 |M_c|^2) / D      (diag terms are 0)
    T_all  = (2 N SQ - 2 |M|^2) / D
    loss   = (2 T_same - T_all) / N^2 + BETA

Device outputs per core, in ONE [128, 264] f32 tensor (single out-DMA):
  out[0:40, 0:256]  = M_c partial sums (one-hot bf16 matmul, PSUM->SBUF)
  out[:, 256:264]   = per-row squared norms rsq[p, k] = |x_{k*128+p}|^2
Host reduces rsq with class-count weights (O(N) bincount) and combines
the 8 partial outputs in float64.

Schedule (per-op costs measured on HW): x arrives in 4 two-chunk DMAs
(more DMA queues in flight = higher aggregate HBM bandwidth; combo rides
early on scalar's queue so the one-hot is ready long before matmul 0).
Per 2-chunk group: DVE casts both chunks bf16 (one batched copy), ACT
Square+accum produces rowsq for the even chunk while DVE squares the odd
chunk; odd rowsq reduces run batched in DVE gaps.  The matmul chain and
the PSUM->SBUF copy chase the casts; everything funnels into one out-DMA.
"""

import numpy as np

import concourse.bacc as bacc
import concourse.bass as bass
import concourse.tile as tile
from concourse import mybir
from concourse.bass_utils import run_bass_kernel_spmd

N, D = 8192, 256
N_CORES = 8
ROWS = N // N_CORES          # 1024 rows per core
P = 128                      # partitions
CHUNKS = ROWS // P           # 8 chunks of 128 rows
NCLS = 40
BETA = 1.0
OUTW = D + CHUNKS            # 264

ACT_CHUNKS = (0, 2, 4, 6)    # rowsq via ACT Square+accum
DVE_CHUNKS = (1, 3, 5, 7)    # rowsq via DVE bf16 square + batched reduce

_CACHE = {}


def _bcast(ap, pos, count):
    """Insert a zero-stride dim of size `count` at free-dim position `pos`."""
    pattern = [list(p) for p in ap.ap]
    pattern.insert(pos, [0, count])
    return bass.AP(tensor=ap.tensor, offset=ap.offset, ap=pattern)


def _hbm_chunks(x, k0, k1):
    """AP over HBM rows [k0*128, k1*128) laid out as [128, k1-k0, D]:
    element (p, k, d) -> x[(k0+k)*128 + p, d]."""
    return bass.AP(
        tensor=x.tensor,
        offset=x.offset + k0 * P * D,
        ap=[[D, P], [P * D, k1 - k0], [1, D]],
    )


def _build_bass():
    nc = bacc.Bacc(
        "TRN2",
        target_bir_lowering=False,
        debug=False,
        enable_asserts=False,
        num_devices=N_CORES,
    )
    x = nc.dram_tensor("x", [ROWS, D], mybir.dt.float32, kind="ExternalInput")
    combo = nc.dram_tensor(
        "combo", [P, NCLS + CHUNKS], mybir.dt.float32, kind="ExternalInput"
    )
    out_t = nc.dram_tensor("out", [P, OUTW], mybir.dt.float32, kind="ExternalOutput")

    with tile.TileContext(nc) as tc:
        with (
            tc.tile_pool(name="work", bufs=1) as work,
            tc.tile_pool(name="psum", bufs=1, space="PSUM") as psum_pool,
        ):
            xf = work.tile([P, CHUNKS, D], mybir.dt.float32, tag="xf")
            xb = work.tile([P, CHUNKS, D], mybir.dt.bfloat16, tag="xb")
            combo_sb = work.tile([P, NCLS + CHUNKS], mybir.dt.float32, tag="combo_sb")
            oh = work.tile([P, CHUNKS, NCLS], mybir.dt.bfloat16, tag="oh")
            # compact bf16 X^2 slab for the DVE chunks (adjacent -> batched
            # reduce over pairs)
            sq_v = work.tile([P, 4, D], mybir.dt.bfloat16, tag="sq_v")
            scr_a = work.tile([P, D], mybir.dt.float32, tag="scr_a")
            out_sb = work.tile([P, OUTW], mybir.dt.float32, tag="out_sb")
            acc = psum_pool.tile([NCLS, D], mybir.dt.float32, tag="acc")

            # input DMAs: x in 4 two-chunk slabs; combo early on scalar
            nc.sync.dma_start(out=xf[:, 0:2, :], in_=_hbm_chunks(x[:, :], 0, 2))
            nc.scalar.dma_start(out=combo_sb, in_=combo[:, :])
            nc.sync.dma_start(out=xf[:, 2:4, :], in_=_hbm_chunks(x[:, :], 2, 4))
            nc.scalar.dma_start(out=xf[:, 4:6, :], in_=_hbm_chunks(x[:, :], 4, 6))
            nc.sync.dma_start(out=xf[:, 6:8, :], in_=_hbm_chunks(x[:, :], 6, 8))

            # one-hot: oh[p, k, c] = (cls[p, k] == c)
            iota_sb = combo_sb[:, :NCLS]
            cls_sb = combo_sb[:, NCLS:]
            nc.vector.tensor_tensor(
                out=oh[:, :, :],
                in0=_bcast(cls_sb, 2, NCLS),
                in1=_bcast(iota_sb, 1, CHUNKS),
                op=mybir.AluOpType.is_equal,
            )

            rsq = out_sb[:, D:OUTW]
            for g in range(4):
                e, o = 2 * g, 2 * g + 1       # even: ACT rowsq, odd: DVE
                nc.vector.tensor_copy(xb[:, e : o + 1, :], xf[:, e : o + 1, :])
                nc.scalar.activation(
                    out=scr_a,
                    in_=xf[:, e, :],
                    func=mybir.ActivationFunctionType.Square,
                    accum_out=rsq[:, e : e + 1],
                )
                nc.vector.tensor_mul(sq_v[:, g, :], xb[:, o, :], xb[:, o, :])
                nc.tensor.matmul(acc, oh[:, e, :], xb[:, e, :], start=(e == 0), stop=False)
                nc.tensor.matmul(acc, oh[:, o, :], xb[:, o, :], start=False, stop=(o == 7))
                if g % 2 == 1:
                    # batched rowsq reduce for the two preceding odd chunks
                    nc.vector.reduce_sum(
                        out=rsq[:, 2 * g - 1 : 2 * g + 2 : 2],
                        in_=sq_v[:, g - 1 : g + 1, :],
                        axis=mybir.AxisListType.X,
                    )

            # PSUM -> SBUF rows 0:40 (DMA cannot read PSUM), single out DMA
            nc.scalar.copy(out_sb[:NCLS, :D], acc[:, :])
            nc.sync.dma_start(out=out_t[:, :], in_=out_sb)

    return nc


def _get_nc():
    if "nc" not in _CACHE:
        nc = _build_bass()
        nc.finalize()
        _CACHE["nc"] = nc
    return _CACHE["nc"]


_IOTA = np.broadcast_to(np.arange(NCLS, dtype=np.float32), (P, NCLS))


def run_device(output, classes, **spmd_kwargs):
    """Run the per-core Bass kernel; returns (list of per-core outputs, results)."""
    x = np.ascontiguousarray(np.asarray(output), dtype=np.float32)
    cls_f = np.asarray(classes).astype(np.float32)
    in_maps = []
    for s in range(N_CORES):
        xs = x[s * ROWS : (s + 1) * ROWS]
        cs = cls_f[s * ROWS : (s + 1) * ROWS]
        combo = np.concatenate([_IOTA, cs.reshape(CHUNKS, P).T], axis=1)
        in_maps.append({"x": xs, "combo": np.ascontiguousarray(combo)})
    res = run_bass_kernel_spmd(
        _get_nc(), in_maps, core_ids=list(range(N_CORES)), **spmd_kwargs
    )
    outs = [res.results[s]["out"] for s in range(N_CORES)]
    return outs, res


def _combine(outs, classes):
    """Combine per-core partial outputs into the scalar loss (float64)."""
    cls = np.asarray(classes).astype(np.int64)
    M_c = np.zeros((NCLS, D), dtype=np.float64)
    rowsq = np.empty(N, dtype=np.float64)
    for s, out in enumerate(outs):
        M_c += out[:NCLS, :D].astype(np.float64)
        rowsq[s * ROWS : (s + 1) * ROWS] = (
            out[:, D:OUTW].astype(np.float64).T.reshape(ROWS)
        )
    n_c = np.bincount(cls, minlength=NCLS).astype(np.float64)
    SQ_c = np.bincount(cls, weights=rowsq, minlength=NCLS)
    SQ = rowsq.sum()
    M = M_c.sum(axis=0)
    T_same = (2.0 * (n_c * SQ_c).sum() - 2.0 * (M_c * M_c).sum()) / D
    T_all = (2.0 * N * SQ - 2.0 * (M @ M)) / D
    loss = (2.0 * T_same - T_all) / (float(N) * float(N)) + BETA
    return np.float32(loss)


def kernel(output, classes):
    outs, _ = run_device(output, classes)
    return _combine(outs, classes)
